# revision 7
# baseline (speedup 1.0000x reference)
"""Multi-head attention (B=4, S=2048, D=1024, H=16, Dk=64) on 8 trn2 NeuronCores.

Sharding: core = (batch b, head-group g) with b in 0..3, g in 0..1.
Each core computes attention for its batch and its 8 heads, plus the partial
out-projection for its 512 columns of Wo.  Host sums the two partials per
batch and adds bo.

Per-core kernel (all matmuls in float32r = full-precision fp32 fast mode):
  phase A: PE-transpose q/k/v 128x128 tiles; project to
           qhT/khT [c=512, s=2048] (c on partitions, pairs of heads per
           128-partition tile) and vh [s=2048, c] stored with a ones column
           per head ([128, 16, 8, 65] layout).
  phase B: per head, per 1024-wide query chunk:
           scoresT[sk,sq] = khT_h^T @ qhT_h  (K=64 contraction)
           probsT = exp(scoresT/8 + maskbias[sk])   (mask folded into the
           ACT per-partition bias; masked keys underflow to exactly 0)
           attnT[c(+sums),sq] += vh_ext^T @ probsT  (ones column gives the
           softmax denominator in row 64 for free)
           normalize: recip of row 64, replicate across 64 partitions with a
           K=1 outer-product matmul, multiply -> concatT
  phase C: out[s, :] = concatT^T @ Wo  (accumulate 4 c-chunks in PSUM)
"""

import os
import sys

sys.path.insert(0, "/opt/trn_rl_repo")

import numpy as np

B, S, D, H, DK = 4, 2048, 1024, 16, 64
CPG = 512          # projection columns per core (8 heads x 64)
NCORES = 8

_cache = {}


def _build_nc():
    import concourse.bass as bass
    import concourse.tile as tile
    from concourse import bacc, mybir
    from concourse.masks import make_identity

    f32 = mybir.dt.float32
    R = mybir.dt.float32r
    Exp = mybir.ActivationFunctionType.Exp

    nc = bacc.Bacc("TRN2", target_bir_lowering=False, debug=False)

    q_d = nc.dram_tensor("q", [S, D], f32, kind="ExternalInput").ap()
    k_d = nc.dram_tensor("k", [S, D], f32, kind="ExternalInput").ap()
    v_d = nc.dram_tensor("v", [S, D], f32, kind="ExternalInput").ap()
    wq_d = nc.dram_tensor("wq", [D, CPG], R, kind="ExternalInput").ap()
    wk_d = nc.dram_tensor("wk", [D, CPG], R, kind="ExternalInput").ap()
    wv_d = nc.dram_tensor("wv", [D, CPG], R, kind="ExternalInput").ap()
    wo_d = nc.dram_tensor("wo", [CPG, D], R, kind="ExternalInput").ap()
    bq_d = nc.dram_tensor("bq", [CPG], R, kind="ExternalInput").ap()
    bk_d = nc.dram_tensor("bk", [CPG], R, kind="ExternalInput").ap()
    bv_d = nc.dram_tensor("bv", [CPG], R, kind="ExternalInput").ap()
    mb_d = nc.dram_tensor("maskbias", [128, 16], f32, kind="ExternalInput").ap()
    ones_d = nc.dram_tensor("ones", [128, 512], R, kind="ExternalInput").ap()
    out_d = nc.dram_tensor("out", [S, D], f32, kind="ExternalOutput").ap()

    NSQ = S // 512       # 4 query/key 512-blocks
    NDCH = D // 128      # 8 contraction chunks for projections
    NSK = S // 128       # 16 key chunks
    NPAIR = 4            # head pairs per core

    with tile.TileContext(nc) as tc:
        import contextlib

        with contextlib.ExitStack() as ctx:
            # ---------- persistent tensors ----------
            persist = ctx.enter_context(tc.tile_pool(name="persist", bufs=1))
            consts = ctx.enter_context(tc.tile_pool(name="consts", bufs=1))

            qhT_sb = persist.tile([128, NPAIR, S], R)   # [c%128, pair, sq]
            khT_sb = persist.tile([128, NPAIR, S], R)
            vh_sb = persist.tile([128, NSK, 8, DK + 1], R)  # ones col at 64
            concatT_sb = persist.tile([128, NPAIR, S], R)

            ident = consts.tile([128, 128], f32)
            make_identity(nc, ident)
            ones_sb = consts.tile([1, 512], R)
            nc.sync.dma_start(out=ones_sb, in_=ones_d[0:1, :])
            nc.sync.dma_start(
                out=vh_sb[:, :, :, DK],
                in_=ones_d[:, 0:128].rearrange("p (a b) -> p a b", a=16),
            )
            mb_sb = consts.tile([128, 16], f32)
            nc.sync.dma_start(out=mb_sb, in_=mb_d)
            bq_sb = consts.tile([1, CPG], R)
            nc.sync.dma_start(out=bq_sb, in_=bq_d[None, :])
            bk_sb = consts.tile([1, CPG], R)
            nc.sync.dma_start(out=bk_sb, in_=bk_d[None, :])
            bv_sb = consts.tile([1, CPG], R)
            nc.sync.dma_start(out=bv_sb, in_=bv_d[None, :])

            # ---------- phase A: transposes + projections ----------
            with contextlib.ExitStack() as actx:
                wpool = actx.enter_context(tc.tile_pool(name="wpool", bufs=2))
                natpool = actx.enter_context(tc.tile_pool(name="natpool", bufs=6))
                xtpool = actx.enter_context(tc.tile_pool(name="xtpool", bufs=3))
                tppool = actx.enter_context(
                    tc.tile_pool(name="tppool", bufs=2, space="PSUM")
                )
                prpool = actx.enter_context(
                    tc.tile_pool(name="prpool", bufs=4, space="PSUM")
                )

                for tname, x_d, w_d, b_sb in (
                    ("q", q_d, wq_d, bq_sb),
                    ("k", k_d, wk_d, bk_sb),
                    ("v", v_d, wv_d, bv_sb),
                ):
                    w_sb = wpool.tile([128, NDCH, CPG], R, tag="w")
                    nc.sync.dma_start(
                        out=w_sb, in_=w_d.rearrange("(j p) c -> p j c", p=128)
                    )
                    for sq in range(NSQ):
                        nats = []
                        for i in range(4):
                            x_nat = natpool.tile([128, D], f32, tag="nat")
                            r0 = sq * 512 + i * 128
                            nc.sync.dma_start(out=x_nat, in_=x_d[r0 : r0 + 128, :])
                            nats.append(x_nat)

                        # open accumulation: bias outer-product first
                        prs = []
                        for cch in range(4):
                            pr = prpool.tile([128, 512], f32, tag="pr")
                            prs.append(pr)
                            if tname == "v":
                                nc.tensor.matmul(
                                    pr,
                                    lhsT=ones_sb[0:1, 0:128],
                                    rhs=b_sb[0:1, :],
                                    start=True,
                                    stop=False,
                                )
                            else:
                                nc.tensor.matmul(
                                    pr,
                                    lhsT=b_sb[0:1, cch * 128 : cch * 128 + 128],
                                    rhs=ones_sb[0:1, 0:512],
                                    start=True,
                                    stop=False,
                                )

                        for j in range(NDCH):
                            tp = tppool.tile([128, 512], f32, tag="tp")
                            for i in range(4):
                                nc.tensor.transpose(
                                    out=tp[:, i * 128 : i * 128 + 128],
                                    in_=nats[i][:, j * 128 : j * 128 + 128],
                                    identity=ident,
                                )
                            xt = xtpool.tile([128, 512], R, tag="xt")
                            nc.scalar.copy(out=xt, in_=tp)
                            for cch in range(4):
                                if tname == "v":
                                    # vh[sk,c]: lhsT = xT chunk, rhs = W chunk
                                    nc.tensor.matmul(
                                        prs[cch],
                                        lhsT=xt[:, cch * 128 : cch * 128 + 128],
                                        rhs=w_sb[:, j, :],
                                        start=False,
                                        stop=(j == NDCH - 1),
                                    )
                                else:
                                    # qhT[c,sq]: lhsT = W chunk, rhs = xT
                                    nc.tensor.matmul(
                                        prs[cch],
                                        lhsT=w_sb[:, j, cch * 128 : cch * 128 + 128],
                                        rhs=xt,
                                        start=False,
                                        stop=(j == NDCH - 1),
                                    )

                        for cch in range(4):
                            if tname == "v":
                                # prs[cch] = vh rows [sq*512+cch*128, +128)
                                skc = sq * 4 + cch
                                nc.vector.tensor_copy(
                                    out=vh_sb[:, skc, :, 0:DK],
                                    in_=prs[cch].rearrange("p (h d) -> p h d", h=8),
                                )
                            else:
                                dst = qhT_sb if tname == "q" else khT_sb
                                nc.vector.tensor_copy(
                                    out=dst[:, cch, sq * 512 : sq * 512 + 512],
                                    in_=prs[cch],
                                )

            # ---------- phase B: attention ----------
            with contextlib.ExitStack() as bctx:
                probpool = bctx.enter_context(tc.tile_pool(name="probpool", bufs=4))
                npool = bctx.enter_context(tc.tile_pool(name="npool", bufs=2))
                rppool = bctx.enter_context(tc.tile_pool(name="rppool", bufs=2))
                scpool = bctx.enter_context(
                    tc.tile_pool(name="scpool", bufs=2, space="PSUM")
                )
                atpool = bctx.enter_context(
                    tc.tile_pool(name="atpool", bufs=1, space="PSUM")
                )
                reppool = bctx.enter_context(
                    tc.tile_pool(name="reppool", bufs=1, space="PSUM")
                )

                for pair in range(NPAIR):
                    for hh in range(2):
                        h = pair * 2 + hh
                        base = hh * 64
                        for sq2 in range(S // 1024):
                            at_ps = atpool.tile([128, 1024], f32, tag="at")
                            for sk in range(NSK):
                                sc_ps = scpool.tile([128, 1024], f32, tag="sc")
                                for half in range(2):
                                    qoff = sq2 * 1024 + half * 512
                                    nc.tensor.matmul(
                                        sc_ps[:, half * 512 : half * 512 + 512],
                                        lhsT=khT_sb[
                                            base : base + 64,
                                            pair,
                                            sk * 128 : sk * 128 + 128,
                                        ],
                                        rhs=qhT_sb[
                                            base : base + 64, pair, qoff : qoff + 512
                                        ],
                                        start=True,
                                        stop=True,
                                    )
                                probs = probpool.tile([128, 1024], R, tag="probs")
                                nc.scalar.activation(
                                    out=probs,
                                    in_=sc_ps,
                                    func=Exp,
                                    bias=mb_sb[:, sk : sk + 1],
                                    scale=0.125,
                                )
                                for half in range(2):
                                    nc.tensor.matmul(
                                        at_ps[0:65, half * 512 : half * 512 + 512],
                                        lhsT=vh_sb[:, sk, h, :],
                                        rhs=probs[:, half * 512 : half * 512 + 512],
                                        start=(sk == 0),
                                        stop=(sk == NSK - 1),
                                    )
                            attn_sb = npool.tile([128, 1024], f32, tag="attn")
                            nc.vector.tensor_copy(
                                out=attn_sb[0:65, :], in_=at_ps[0:65, :]
                            )
                            recip32 = rppool.tile([1, 1024], f32, tag="recip32")
                            nc.vector.reciprocal(recip32, attn_sb[64:65, :])
                            recip = rppool.tile([1, 1024], R, tag="recip")
                            nc.vector.tensor_copy(out=recip, in_=recip32)
                            rep_ps = reppool.tile([64, 1024], f32, tag="rep")
                            for half in range(2):
                                nc.tensor.matmul(
                                    rep_ps[:, half * 512 : half * 512 + 512],
                                    lhsT=ones_sb[0:1, 0:64],
                                    rhs=recip[0:1, half * 512 : half * 512 + 512],
                                    start=True,
                                    stop=True,
                                )
                            nc.vector.tensor_mul(
                                concatT_sb[
                                    base : base + 64,
                                    pair,
                                    sq2 * 1024 : sq2 * 1024 + 1024,
                                ],
                                attn_sb[0:64, :],
                                rep_ps,
                            )

            # ---------- phase C: out projection ----------
            with contextlib.ExitStack() as cctx:
                wopool = cctx.enter_context(tc.tile_pool(name="wopool", bufs=1))
                outpool = cctx.enter_context(tc.tile_pool(name="outpool", bufs=3))
                opspool = cctx.enter_context(
                    tc.tile_pool(name="opspool", bufs=4, space="PSUM")
                )

                wo_sb = wopool.tile([128, NPAIR, D], R)
                nc.sync.dma_start(
                    out=wo_sb, in_=wo_d.rearrange("(j p) c -> p j c", p=128)
                )
                for sqc in range(S // 128):
                    for do in range(2):
                        o_ps = opspool.tile([128, 512], f32, tag="ops")
                        for j in range(NPAIR):
                            nc.tensor.matmul(
                                o_ps,
                                lhsT=concatT_sb[:, j, sqc * 128 : sqc * 128 + 128],
                                rhs=wo_sb[:, j, do * 512 : do * 512 + 512],
                                start=(j == 0),
                                stop=(j == NPAIR - 1),
                            )
                        o_sb = outpool.tile([128, 512], f32, tag="osb")
                        nc.vector.tensor_copy(out=o_sb, in_=o_ps)
                        nc.sync.dma_start(
                            out=out_d[
                                sqc * 128 : sqc * 128 + 128, do * 512 : do * 512 + 512
                            ],
                            in_=o_sb,
                        )

    nc.compile()
    return nc


def get_nc():
    if "nc" not in _cache:
        _cache["nc"] = _build_nc()
    return _cache["nc"]


def make_in_maps(q, k, v, mask, Wq, bq, Wk, bk, Wv, bv, Wo, bo):
    f32 = np.float32
    c = np.ascontiguousarray
    in_maps = []
    for core in range(NCORES):
        b, g = core // 2, core % 2
        cols = slice(g * CPG, (g + 1) * CPG)
        mb = (-1e9 * (1.0 - np.asarray(mask[b, 0], f32))).reshape(16, 128).T
        in_maps.append(
            {
                "q": c(np.asarray(q[b], f32)),
                "k": c(np.asarray(k[b], f32)),
                "v": c(np.asarray(v[b], f32)),
                "wq": c(np.asarray(Wq[:, cols], f32)),
                "wk": c(np.asarray(Wk[:, cols], f32)),
                "wv": c(np.asarray(Wv[:, cols], f32)),
                "wo": c(np.asarray(Wo[cols, :], f32)),
                "bq": c(np.asarray(bq[cols], f32)),
                "bk": c(np.asarray(bk[cols], f32)),
                "bv": c(np.asarray(bv[cols], f32)),
                "maskbias": c(mb),
                "ones": np.ones((128, 512), f32),
            }
        )
    return in_maps


def gather(results, bo):
    out = np.zeros((B, S, D), np.float32)
    for core in range(NCORES):
        b = core // 2
        out[b] += results[core]["out"]
    out += np.asarray(bo, np.float32)[None, None, :]
    return out


def run_on_hw(in_maps, trace=False, trace_cores=None):
    from concourse.bass_utils import run_bass_kernel_spmd

    nc = get_nc()
    return run_bass_kernel_spmd(
        nc,
        in_maps,
        list(range(NCORES)),
        trace=trace,
        trace_cores=trace_cores,
    )


def kernel(q, k, v, mask, Wq, bq, Wk, bk, Wv, bv, Wo, bo):
    in_maps = make_in_maps(q, k, v, mask, Wq, bq, Wk, bk, Wv, bv, Wo, bo)
    res = run_on_hw(in_maps)
    return gather(res.results, bo)
